# revision 7
# baseline (speedup 1.0000x reference)
"""GCN (2-layer) + global mean pool on 8 Trainium2 NeuronCores.

Strategy
--------
Nodes are padded to 100352 = 784 tiles of 128; dest tiles are split
contiguously across 8 cores (98 each).  Per layer the replicated fp16 table

    g[n] = dinv[n] * h[n]        ([100352, 128], AllGather per layer)

is gathered per edge with the GPSIMD `dma_gather` custom DMA (int16 indices,
4 SWDGE queues in parallel).  int16 only reaches 65536 rows, so rows are
addressed in 512-byte pairs with a signed-window trick: the AP base sits at
pair 32768 and idx = node//2 - 32768 (parity picks the 256B half); edges are
split per dest tile into even/odd-source streams so every 2048-index call is
parity-pure.

Edges sorted by destination are processed in 128-edge chunks: the 0/1
selection matrix S[e, c] = (lid[e] == c) is built on the vector engine from a
tiled iota; aggregation is a PE matmul accumulated per dest-tile in PSUM.
deg^-1/2 is folded into the table (source side) and activation scales /
K=1 bias matmuls (dest side), so no per-edge multiply exists anywhere.

Layer 1 stores relu(dinv^2*agg + dinv*b1) as the next table; layer 2
aggregates transposed (lhsT=msg, rhs=S) so W2 consumes the aggregate as
stationary operand; mean pooling is one more indicator matmul accumulated
over all tiles in PSUM; the host sums the 8 per-core [64,128] partials.
"""

import sys

if '/opt/trn_rl_repo' not in sys.path:
    sys.path.insert(0, '/opt/trn_rl_repo')

import numpy as np

NCORES = 8
N = 100000
NP = 100352          # 784 tiles of 128
NTILES = 784
TPC = NTILES // NCORES   # 98 dest tiles per core
NPC = TPC * 128          # 12544 nodes per core
D = 512
H = 128
NG = 64
CPC = 16             # chunks per gather call / S-build batch (2048 idxs)

_cache = {}


def _wrap_idx(vals16, tc_pad):
    """[128, tc_pad] int16 per-(lane, chunk) values -> dma_gather wrapped
    layout [128, tc_pad*8]: within-call index i = (q%16)*128 + p lives at
    [p%16 (+16r), q*8 + p//16]."""
    lanes = np.arange(128)
    out16 = np.zeros((16, tc_pad * 8), np.int16)
    cols = (lanes[:, None] // 16) + np.arange(tc_pad)[None, :] * 8
    rows = (lanes % 16)[:, None].repeat(tc_pad, axis=1)
    out16[rows, cols] = vals16
    return np.tile(out16, (8, 1))


def _host_prep_graph(edge_index, batch):
    ei = np.asarray(edge_index)
    row = np.concatenate([ei[0], np.arange(N, dtype=np.int64)])
    col = np.concatenate([ei[1], np.arange(N, dtype=np.int64)])
    deg = np.bincount(col, minlength=N).astype(np.float64)
    dinv = 1.0 / np.sqrt(deg)

    # sort by (dest tile, src-parity): per-(tile, parity) contiguous segments
    key = (col >> 7) * 2 + (row & 1)
    order = np.argsort(key, kind='stable')
    row_s = row[order].astype(np.int64)
    col_s = col[order].astype(np.int64)
    par_s = (row_s & 1).astype(np.int64)

    tile_of = col_s >> 7
    # counts per (tile, parity)
    tp = tile_of * 2 + par_s
    tp_cnt = np.bincount(tp, minlength=NTILES * 2).reshape(NTILES, 2)
    tp_start = np.zeros(NTILES * 2 + 1, np.int64)
    np.cumsum(tp_cnt.reshape(-1), out=tp_start[1:])
    tp_start = tp_start[:-1].reshape(NTILES, 2)

    cntE = tp_cnt[:, 0].reshape(NCORES, TPC)
    cntO = tp_cnt[:, 1].reshape(NCORES, TPC)
    cslotE = np.ceil(cntE / 128).astype(np.int64).max(axis=0)
    cslotO = np.ceil(cntO / 128).astype(np.int64).max(axis=0)

    def pad16(x):
        return ((x + CPC - 1) // CPC) * CPC

    tcE, tcO = int(cslotE.sum()), int(cslotO.sum())
    tcE_pad, tcO_pad = pad16(max(tcE, 1)), pad16(max(tcO, 1))

    baseE = np.zeros(TPC + 1, np.int64)
    np.cumsum(cslotE, out=baseE[1:])
    baseO = np.zeros(TPC + 1, np.int64)
    np.cumsum(cslotO, out=baseO[1:])

    base_pair = 32768 if NP // 2 > 32768 else 0
    idxs = {0: np.zeros((NCORES, 128, tcE_pad), np.int16),
            1: np.zeros((NCORES, 128, tcO_pad), np.int16)}
    lids = {0: np.full((NCORES, 128, tcE_pad), -1.0, np.float16),
            1: np.full((NCORES, 128, tcO_pad), -1.0, np.float16)}
    bases = {0: baseE, 1: baseO}
    for c in range(NCORES):
        for i in range(TPC):
            t = c * TPC + i
            for h in (0, 1):
                s, n = tp_start[t, h], tp_cnt[t, h]
                if n == 0:
                    continue
                src = row_s[s:s + n]
                loc = (col_s[s:s + n] - (t << 7)).astype(np.float16)
                j0 = bases[h][i]
                ch = np.arange(n) // 128 + j0
                ln = np.arange(n) % 128
                idxs[h][c, ln, ch] = (src >> 1) - base_pair
                lids[h][c, ln, ch] = loc

    # the Q7 strips trailing negative indices per call: make sure the last
    # index (lane 127 of the last chunk) of every 16-chunk call is >= 0 by
    # swapping lanes inside that chunk (S/lid follow the same permutation).
    for h in (0, 1):
        tcp = idxs[h].shape[2]
        for c in range(NCORES):
            for c0 in range(0, tcp, CPC):
                col = c0 + CPC - 1
                if idxs[h][c, 127, col] < 0:
                    ok = np.nonzero(idxs[h][c, :, col] >= 0)[0]
                    assert len(ok), "all-negative chunk"
                    p = ok[0]
                    for arr in (idxs[h], lids[h]):
                        arr[c, [127, p], col] = arr[c, [p, 127], col]

    idxE_w = np.stack([_wrap_idx(idxs[0][c], tcE_pad) for c in range(NCORES)])
    idxO_w = np.stack([_wrap_idx(idxs[1][c], tcO_pad) for c in range(NCORES)])

    dinv_p = np.ones(NP, np.float64)
    dinv_p[:N] = dinv
    rdinv = np.zeros(NP, np.float16)
    rdinv[:N] = (1.0 / dinv).astype(np.float16)
    dinv2 = (dinv_p ** 2).astype(np.float32)
    dinv1 = dinv_p.astype(np.float32)

    b = np.asarray(batch).astype(np.int64)
    cnt_g = np.bincount(b, minlength=NG).astype(np.float64)
    invcnt = 1.0 / np.maximum(cnt_g, 1.0)
    blid = np.full(NP, -1.0, np.float32)
    blid[:N] = b.astype(np.float32)
    invc = np.zeros(NP, np.float32)
    invc[:N] = invcnt[b].astype(np.float32)

    return dict(
        dinv=dinv, cslotE=cslotE, cslotO=cslotO,
        tcE_pad=tcE_pad, tcO_pad=tcO_pad,
        idxE=idxE_w, idxO=idxO_w, lidE=lids[0], lidO=lids[1],
        base_pair=base_pair,
        rdinv=rdinv, dinv2=dinv2, dinv1=dinv1, blid=blid, invc=invc,
    )


def _build_nc(gp):
    import concourse.bass as bass
    import concourse.bacc as bacc
    import concourse.mybir as mybir
    import concourse.tile as tile

    fp16 = mybir.dt.float16
    fp32 = mybir.dt.float32
    i16 = mybir.dt.int16
    Relu = mybir.ActivationFunctionType.Relu
    iseq = mybir.AluOpType.is_equal
    mult = mybir.AluOpType.mult

    tcE_pad, tcO_pad = gp['tcE_pad'], gp['tcO_pad']
    cslotE, cslotO = gp['cslotE'], gp['cslotO']
    BASE_PAIR = gp['base_pair']
    UHALF = NP // 2 - BASE_PAIR   # pair-rows above the window base

    nc = bacc.Bacc("TRN2", target_bir_lowering=False, debug=False,
                   num_devices=NCORES, num_swdge_queues=4)

    xT_in = nc.dram_tensor("xT", [D, NPC], fp16, kind="ExternalInput").ap()
    idxE_in = nc.dram_tensor("idxE", [128, tcE_pad * 8], i16,
                             kind="ExternalInput").ap()
    idxO_in = nc.dram_tensor("idxO", [128, tcO_pad * 8], i16,
                             kind="ExternalInput").ap()
    lidE_in = nc.dram_tensor("lidE", [128, tcE_pad], fp16,
                             kind="ExternalInput").ap()
    lidO_in = nc.dram_tensor("lidO", [128, tcO_pad], fp16,
                             kind="ExternalInput").ap()
    w1_in = nc.dram_tensor("w1", [128, 4 * 128], fp16, kind="ExternalInput").ap()
    w2_in = nc.dram_tensor("w2", [128, 128], fp16, kind="ExternalInput").ap()
    b1_in = nc.dram_tensor("b1", [1, 128], fp16, kind="ExternalInput").ap()
    b2_in = nc.dram_tensor("b2", [1, 128], fp16, kind="ExternalInput").ap()
    rdinv_in = nc.dram_tensor("rdinv", [1, NPC], fp16, kind="ExternalInput").ap()
    dinv2_in = nc.dram_tensor("dinv2", [128, TPC], fp32, kind="ExternalInput").ap()
    dinv1_in = nc.dram_tensor("dinv1", [128, TPC], fp32, kind="ExternalInput").ap()
    blid_in = nc.dram_tensor("blid", [128, TPC], fp32, kind="ExternalInput").ap()
    invc_in = nc.dram_tensor("invc", [128, TPC], fp32, kind="ExternalInput").ap()
    iota_in = nc.dram_tensor("iota", [128, CPC * 128], fp16,
                             kind="ExternalInput").ap()
    iota64_in = nc.dram_tensor("iota64", [128, 64], fp16,
                               kind="ExternalInput").ap()
    out_dram = nc.dram_tensor("pooled", [64, 128], fp32,
                              kind="ExternalOutput").ap()

    with tile.TileContext(nc) as tcx:
        import contextlib
        ctx = contextlib.ExitStack()
        with ctx:
            dram = ctx.enter_context(tcx.tile_pool(name="dram", bufs=1, space="DRAM"))
            cpool = ctx.enter_context(tcx.tile_pool(name="const", bufs=1))
            xt_pool = ctx.enter_context(tcx.tile_pool(name="xt", bufs=6))
            g0sb_pool = ctx.enter_context(tcx.tile_pool(name="g0sb", bufs=4))
            msg_pool = ctx.enter_context(tcx.tile_pool(name="msg", bufs=6))
            s_pool = ctx.enter_context(tcx.tile_pool(name="spool", bufs=6))
            h1sb_pool = ctx.enter_context(tcx.tile_pool(name="h1sb", bufs=4))
            aggt_pool = ctx.enter_context(tcx.tile_pool(name="aggt", bufs=4))
            h2sb_pool = ctx.enter_context(tcx.tile_pool(name="h2sb", bufs=4))
            p_pool = ctx.enter_context(tcx.tile_pool(name="ppool", bufs=4))
            osb_pool = ctx.enter_context(tcx.tile_pool(name="osb", bufs=1))
            psA = ctx.enter_context(tcx.tile_pool(name="psA", bufs=4, space="PSUM"))
            psB = ctx.enter_context(tcx.tile_pool(name="psB", bufs=2, space="PSUM"))
            psP = ctx.enter_context(tcx.tile_pool(name="psP", bufs=1, space="PSUM"))

            g0_local = dram.tile([NPC, 128], fp16)
            g0_full = dram.tile([NP, 128], fp16, addr_space="Shared")
            g1_local = dram.tile([NPC, 128], fp16)
            g1_full = dram.tile([NP, 128], fp16, addr_space="Shared")

            # ---- load constants ----
            def cload(name, ap_in, shape, dt):
                t = cpool.tile(shape, dt, name=name)
                nc.sync.dma_start(out=t[:], in_=ap_in)
                return t

            idxE_sb = cload("idxE_sb", idxE_in, [128, tcE_pad * 8], i16)
            idxO_sb = cload("idxO_sb", idxO_in, [128, tcO_pad * 8], i16)
            lidE_sb = cload("lidE_sb", lidE_in, [128, tcE_pad], fp16)
            lidO_sb = cload("lidO_sb", lidO_in, [128, tcO_pad], fp16)
            w1_sb = cload("w1_sb", w1_in, [128, 4 * 128], fp16)
            w2_sb = cload("w2_sb", w2_in, [128, 128], fp16)
            b1_sb = cload("b1_sb", b1_in, [1, 128], fp16)
            b2_sb = cload("b2_sb", b2_in, [1, 128], fp16)
            rdinv_sb = cload("rdinv_sb", rdinv_in, [1, NPC], fp16)
            dinv2_sb = cload("dinv2_sb", dinv2_in, [128, TPC], fp32)
            dinv1_sb = cload("dinv1_sb", dinv1_in, [128, TPC], fp32)
            blid_sb = cload("blid_sb", blid_in, [128, TPC], fp32)
            invc_sb = cload("invc_sb", invc_in, [128, TPC], fp32)
            iota_sb = cload("iota_sb", iota_in, [128, CPC * 128], fp16)
            iota64_sb = cload("iota64_sb", iota64_in, [128, 64], fp16)

            # ---- g0 = (dinv*x) @ W1, node-sharded ----
            for i in range(TPC):
                ps = psA.tile([128, 128], fp32, tag='agg', name=f'g0ps_{i}')
                for kk in range(4):
                    xt = xt_pool.tile([128, 128], fp16)
                    nc.sync.dma_start(
                        out=xt[:],
                        in_=xT_in[kk * 128:(kk + 1) * 128, i * 128:(i + 1) * 128])
                    nc.tensor.matmul(ps[:], lhsT=xt[:],
                                     rhs=w1_sb[:, kk * 128:(kk + 1) * 128],
                                     start=(kk == 0), stop=(kk == 3))
                g0t = g0sb_pool.tile([128, 128], fp16)
                nc.scalar.copy(out=g0t[:], in_=ps[:])
                nc.sync.dma_start(out=g0_local[i * 128:(i + 1) * 128, :],
                                  in_=g0t[:])

            nc.gpsimd.collective_compute(
                "AllGather", mybir.AluOpType.bypass,
                replica_groups=[list(range(NCORES))],
                ins=[g0_local.opt()], outs=[g0_full.opt()])

            # ---- two GCN layers ----
            for layer in (1, 2):
                src_full = g0_full if layer == 1 else g1_full
                # pair view: [NP/2, 256]; window base at pair 32768
                gv = src_full[:].rearrange("(u two) d -> u (two d)", two=2)
                in_ap = {0: gv[BASE_PAIR:BASE_PAIR + UHALF, 0:128],
                         1: gv[BASE_PAIR:BASE_PAIR + UHALF, 128:256]}
                idx_sb = {0: idxE_sb, 1: idxO_sb}
                lid_sb = {0: lidE_sb, 1: lidO_sb}

                pos = {0: 0, 1: 0}            # next chunk per stream
                tiles_cur = {0: None, 1: None}
                state = {'cc': 0}
                pool_ps = None
                if layer == 2:
                    pool_ps = psP.tile([64, 128], fp32, name='poolps')

                def next_chunk(h, layer=layer, in_ap=in_ap, idx_sb=idx_sb,
                               lid_sb=lid_sb, pos=pos, tiles_cur=tiles_cur,
                               state=state):
                    p = pos[h]
                    if tiles_cur[h] is None or p % CPC == 0:
                        c0 = (p // CPC) * CPC
                        msg = msg_pool.tile([128, CPC * 128], fp16, tag='msg',
                                            name=f'msg_{layer}_{h}_{c0}')
                        nc.gpsimd.dma_gather(
                            msg[:].rearrange("p (k c) -> p k c", c=128),
                            in_ap[h],
                            idx_sb[h][:, c0 * 8:(c0 + CPC) * 8],
                            CPC * 128, CPC * 128, 128,
                            elem_step=256, single_packet=False,
                            queue_num=state['cc'] % 4)
                        state['cc'] += 1
                        sbt = s_pool.tile([128, CPC * 128], fp16, tag='s',
                                          name=f's_{layer}_{h}_{c0}')
                        nc.vector.tensor_tensor(
                            out=sbt[:].rearrange("p (k c) -> p k c", c=128),
                            in0=lid_sb[h][:, c0:c0 + CPC].to_broadcast(
                                [128, CPC, 128]),
                            in1=iota_sb[:].rearrange("p (k c) -> p k c", c=128),
                            op=iseq)
                        tiles_cur[h] = (msg, sbt, c0)
                    pos[h] = p + 1
                    msg, sbt, c0 = tiles_cur[h]
                    jj = p - c0
                    return (msg[:, jj * 128:(jj + 1) * 128],
                            sbt[:, jj * 128:(jj + 1) * 128])

                for i in range(TPC):
                    ce, co = int(cslotE[i]), int(cslotO[i])
                    ctot = ce + co
                    agg_ps = psA.tile([128, 128], fp32, tag='agg',
                                      name=f'agg_{layer}_{i}')
                    for k in range(ctot):
                        h = 0 if k < ce else 1
                        m_ap, s_ap = next_chunk(h)
                        if layer == 1:
                            nc.tensor.matmul(agg_ps[:], lhsT=s_ap, rhs=m_ap,
                                             start=(k == 0), stop=False)
                        else:
                            nc.tensor.matmul(agg_ps[:], lhsT=m_ap, rhs=s_ap,
                                             start=(k == 0),
                                             stop=(k == ctot - 1))
                    # ---- tile epilogue ----
                    rd = rdinv_sb[0:1, i * 128:(i + 1) * 128]
                    if layer == 1:
                        nc.tensor.matmul(agg_ps[:], lhsT=rd, rhs=b1_sb[0:1, :],
                                         start=(ctot == 0), stop=True)
                        h1t = h1sb_pool.tile([128, 128], fp16)
                        nc.scalar.activation(
                            out=h1t[:], in_=agg_ps[:], func=Relu,
                            scale=dinv2_sb[:, i:i + 1])
                        nc.sync.dma_start(
                            out=g1_local[i * 128:(i + 1) * 128, :], in_=h1t[:])
                    else:
                        aggt = aggt_pool.tile([128, 128], fp16)
                        nc.scalar.copy(out=aggt[:], in_=agg_ps[:])
                        h2ps = psB.tile([128, 128], fp32, tag='h2',
                                        name=f'h2ps_{i}')
                        nc.tensor.matmul(h2ps[:], lhsT=aggt[:], rhs=w2_sb[:],
                                         start=True, stop=False)
                        nc.tensor.matmul(h2ps[:], lhsT=rd, rhs=b2_sb[0:1, :],
                                         start=False, stop=True)
                        h2t = h2sb_pool.tile([128, 128], fp16)
                        nc.scalar.activation(
                            out=h2t[:], in_=h2ps[:], func=Relu,
                            scale=dinv1_sb[:, i:i + 1])
                        pt = p_pool.tile([128, 64], fp16)
                        nc.vector.tensor_scalar(
                            out=pt[:], in0=iota64_sb[:],
                            scalar1=blid_sb[:, i:i + 1],
                            scalar2=invc_sb[:, i:i + 1],
                            op0=iseq, op1=mult)
                        nc.tensor.matmul(pool_ps[:], lhsT=pt[:], rhs=h2t[:],
                                         start=(i == 0), stop=(i == TPC - 1))
                if layer == 1:
                    nc.gpsimd.collective_compute(
                        "AllGather", mybir.AluOpType.bypass,
                        replica_groups=[list(range(NCORES))],
                        ins=[g1_local.opt()], outs=[g1_full.opt()])

            pooled_t = osb_pool.tile([64, 128], fp32)
            nc.scalar.copy(out=pooled_t[:], in_=pool_ps[:])
            nc.sync.dma_start(out=out_dram, in_=pooled_t[:])

    nc.compile()
    return nc


def _make_in_maps(inputs, gp):
    x = np.asarray(inputs['x'])
    W1 = np.asarray(inputs['W1'])
    b1 = np.asarray(inputs['b1'])
    W2 = np.asarray(inputs['W2'])
    b2 = np.asarray(inputs['b2'])
    dinv = gp['dinv']

    xs = np.zeros((NP, D), np.float16)
    xs[:N] = (x.astype(np.float64) * dinv[:, None]).astype(np.float16)
    w1r = np.ascontiguousarray(
        W1.astype(np.float16).reshape(4, 128, 128).transpose(1, 0, 2)
    ).reshape(128, 4 * 128)
    w2r = W2.astype(np.float16)
    b1r = b1.astype(np.float16).reshape(1, 128)
    b2r = b2.astype(np.float16).reshape(1, 128)
    iota = np.tile(np.arange(128, dtype=np.float16)[None, :], (128, CPC))
    iota64 = np.tile(np.arange(64, dtype=np.float16)[None, :], (128, 1))

    in_maps = []
    for c in range(NCORES):
        lo, hi = c * NPC, (c + 1) * NPC
        xT = np.ascontiguousarray(xs[lo:hi].T)
        in_maps.append({
            "xT": xT,
            "idxE": gp['idxE'][c], "idxO": gp['idxO'][c],
            "lidE": gp['lidE'][c], "lidO": gp['lidO'][c],
            "w1": w1r, "w2": w2r, "b1": b1r, "b2": b2r,
            "rdinv": gp['rdinv'][lo:hi].reshape(1, NPC),
            "dinv2": gp['dinv2'][lo:hi].reshape(TPC, 128).T.copy(),
            "dinv1": gp['dinv1'][lo:hi].reshape(TPC, 128).T.copy(),
            "blid": gp['blid'][lo:hi].reshape(TPC, 128).T.copy(),
            "invc": gp['invc'][lo:hi].reshape(TPC, 128).T.copy(),
            "iota": iota, "iota64": iota64,
        })
    return in_maps


def _get_built(inputs):
    ei = np.asarray(inputs['edge_index'])
    key = hash((ei.shape, ei[0, :50].tobytes(), ei[1, -50:].tobytes()))
    if _cache.get('key') != key:
        gp = _host_prep_graph(inputs['edge_index'], inputs['batch'])
        nc = _build_nc(gp)
        _cache.update(key=key, gp=gp, nc=nc)
    return _cache['nc'], _cache['gp']


def kernel(run_kwargs=None, **inputs):
    from concourse.bass_utils import run_bass_kernel_spmd
    nc, gp = _get_built(inputs)
    in_maps = _make_in_maps(inputs, gp)
    res = run_bass_kernel_spmd(nc, in_maps, list(range(NCORES)),
                               **(run_kwargs or {}))
    out = np.zeros((64, 128), np.float64)
    for r in res.results:
        out += r["pooled"].astype(np.float64)
    if run_kwargs:
        _cache['last_res'] = res
    return out.astype(np.float32)


# revision 8
# speedup vs baseline: 1.2770x; 1.2770x over previous
"""GCN (2-layer) + global mean pool on 8 Trainium2 NeuronCores.

Strategy
--------
Nodes are padded to 100352 = 784 tiles of 128; dest tiles are split
contiguously across 8 cores (98 each).  Per layer the replicated fp16 table

    g[n] = dinv[n] * h[n]        ([100352, 128], AllGather per layer)

is gathered per edge with the GPSIMD `dma_gather` custom DMA (int16 indices,
4 SWDGE queues in parallel).  int16 only reaches 65536 rows, so rows are
addressed in 512-byte pairs with a signed-window trick: the AP base sits at
pair 32768 and idx = node//2 - 32768 (parity picks the 256B half); edges are
split per dest tile into even/odd-source streams so every 2048-index call is
parity-pure.

Edges sorted by destination are processed in 128-edge chunks: the 0/1
selection matrix S[e, c] = (lid[e] == c) is built on the vector engine from a
tiled iota; aggregation is a PE matmul accumulated per dest-tile in PSUM.
deg^-1/2 is folded into the table (source side) and activation scales /
K=1 bias matmuls (dest side), so no per-edge multiply exists anywhere.

Layer 1 stores relu(dinv^2*agg + dinv*b1) as the next table; layer 2
aggregates transposed (lhsT=msg, rhs=S) so W2 consumes the aggregate as
stationary operand; mean pooling is one more indicator matmul accumulated
over all tiles in PSUM; the host sums the 8 per-core [64,128] partials.
"""

import sys

if '/opt/trn_rl_repo' not in sys.path:
    sys.path.insert(0, '/opt/trn_rl_repo')

import numpy as np

NCORES = 8
N = 100000
NP = 100352          # 784 tiles of 128
NTILES = 784
TPC = NTILES // NCORES   # 98 dest tiles per core
NPC = TPC * 128          # 12544 nodes per core
D = 512
H = 128
NG = 64
CPC = 16             # chunks per gather call / S-build batch (2048 idxs)

_cache = {}


def _wrap_idx(vals16, tc_pad):
    """[128, tc_pad] int16 per-(lane, chunk) values -> dma_gather wrapped
    layout [128, tc_pad*8]: within-call index i = (q%16)*128 + p lives at
    [p%16 (+16r), q*8 + p//16]."""
    lanes = np.arange(128)
    out16 = np.zeros((16, tc_pad * 8), np.int16)
    cols = (lanes[:, None] // 16) + np.arange(tc_pad)[None, :] * 8
    rows = (lanes % 16)[:, None].repeat(tc_pad, axis=1)
    out16[rows, cols] = vals16
    return np.tile(out16, (8, 1))


def _host_prep_graph(edge_index, batch):
    ei = np.asarray(edge_index)
    row = np.concatenate([ei[0], np.arange(N, dtype=np.int64)])
    col = np.concatenate([ei[1], np.arange(N, dtype=np.int64)])
    deg = np.bincount(col, minlength=N).astype(np.float64)
    dinv = 1.0 / np.sqrt(deg)

    # sort by (dest tile, src-parity): per-(tile, parity) contiguous segments
    key = (col >> 7) * 2 + (row & 1)
    order = np.argsort(key, kind='stable')
    row_s = row[order].astype(np.int64)
    col_s = col[order].astype(np.int64)
    par_s = (row_s & 1).astype(np.int64)

    tile_of = col_s >> 7
    # counts per (tile, parity)
    tp = tile_of * 2 + par_s
    tp_cnt = np.bincount(tp, minlength=NTILES * 2).reshape(NTILES, 2)
    tp_start = np.zeros(NTILES * 2 + 1, np.int64)
    np.cumsum(tp_cnt.reshape(-1), out=tp_start[1:])
    tp_start = tp_start[:-1].reshape(NTILES, 2)

    cntE = tp_cnt[:, 0].reshape(NCORES, TPC)
    cntO = tp_cnt[:, 1].reshape(NCORES, TPC)
    cslotE = np.ceil(cntE / 128).astype(np.int64).max(axis=0)
    cslotO = np.ceil(cntO / 128).astype(np.int64).max(axis=0)

    def pad16(x):
        return ((x + CPC - 1) // CPC) * CPC

    tcE, tcO = int(cslotE.sum()), int(cslotO.sum())
    tcE_pad, tcO_pad = pad16(max(tcE, 1)), pad16(max(tcO, 1))

    baseE = np.zeros(TPC + 1, np.int64)
    np.cumsum(cslotE, out=baseE[1:])
    baseO = np.zeros(TPC + 1, np.int64)
    np.cumsum(cslotO, out=baseO[1:])

    base_pair = 32768 if NP // 2 > 32768 else 0
    idxs = {0: np.zeros((NCORES, 128, tcE_pad), np.int16),
            1: np.zeros((NCORES, 128, tcO_pad), np.int16)}
    lids = {0: np.full((NCORES, 128, tcE_pad), -1.0, np.float16),
            1: np.full((NCORES, 128, tcO_pad), -1.0, np.float16)}
    bases = {0: baseE, 1: baseO}
    for c in range(NCORES):
        for i in range(TPC):
            t = c * TPC + i
            for h in (0, 1):
                s, n = tp_start[t, h], tp_cnt[t, h]
                if n == 0:
                    continue
                src = row_s[s:s + n]
                loc = (col_s[s:s + n] - (t << 7)).astype(np.float16)
                j0 = bases[h][i]
                ch = np.arange(n) // 128 + j0
                ln = np.arange(n) % 128
                idxs[h][c, ln, ch] = (src >> 1) - base_pair
                lids[h][c, ln, ch] = loc

    # the Q7 strips trailing negative indices per call: make sure the last
    # index (lane 127 of the last chunk) of every 16-chunk call is >= 0 by
    # swapping lanes inside that chunk (S/lid follow the same permutation).
    for h in (0, 1):
        tcp = idxs[h].shape[2]
        for c in range(NCORES):
            for c0 in range(0, tcp, CPC):
                col = c0 + CPC - 1
                if idxs[h][c, 127, col] < 0:
                    ok = np.nonzero(idxs[h][c, :, col] >= 0)[0]
                    assert len(ok), "all-negative chunk"
                    p = ok[0]
                    for arr in (idxs[h], lids[h]):
                        arr[c, [127, p], col] = arr[c, [p, 127], col]

    idxE_w = np.stack([_wrap_idx(idxs[0][c], tcE_pad) for c in range(NCORES)])
    idxO_w = np.stack([_wrap_idx(idxs[1][c], tcO_pad) for c in range(NCORES)])

    dinv_p = np.ones(NP, np.float64)
    dinv_p[:N] = dinv
    rdinv = np.zeros(NP, np.float16)
    rdinv[:N] = (1.0 / dinv).astype(np.float16)
    dinv2 = (dinv_p ** 2).astype(np.float32)
    dinv1 = dinv_p.astype(np.float32)

    b = np.asarray(batch).astype(np.int64)
    cnt_g = np.bincount(b, minlength=NG).astype(np.float64)
    invcnt = 1.0 / np.maximum(cnt_g, 1.0)
    # pooling indicator, host-built: P[node, g] = (batch[node]==g)/cnt_g
    pmat = np.zeros((NP, NG), np.float16)
    pmat[np.arange(N), b] = invcnt[b].astype(np.float16)

    return dict(
        dinv=dinv, cslotE=cslotE, cslotO=cslotO,
        tcE_pad=tcE_pad, tcO_pad=tcO_pad,
        idxE=idxE_w, idxO=idxO_w, lidE=lids[0], lidO=lids[1],
        base_pair=base_pair,
        rdinv=rdinv, dinv2=dinv2, dinv1=dinv1, pmat=pmat,
    )


def _build_nc(gp):
    import concourse.bass as bass
    import concourse.bacc as bacc
    import concourse.mybir as mybir
    import concourse.tile as tile

    fp16 = mybir.dt.float16
    fp32 = mybir.dt.float32
    i16 = mybir.dt.int16
    Relu = mybir.ActivationFunctionType.Relu
    iseq = mybir.AluOpType.is_equal
    mult = mybir.AluOpType.mult

    tcE_pad, tcO_pad = gp['tcE_pad'], gp['tcO_pad']
    cslotE, cslotO = gp['cslotE'], gp['cslotO']
    BASE_PAIR = gp['base_pair']
    UHALF = NP // 2 - BASE_PAIR   # pair-rows above the window base

    nc = bacc.Bacc("TRN2", target_bir_lowering=False, debug=False,
                   num_devices=NCORES, num_swdge_queues=4)

    xT_in = nc.dram_tensor("xT", [D, NPC], fp16, kind="ExternalInput").ap()
    idxE_in = nc.dram_tensor("idxE", [128, tcE_pad * 8], i16,
                             kind="ExternalInput").ap()
    idxO_in = nc.dram_tensor("idxO", [128, tcO_pad * 8], i16,
                             kind="ExternalInput").ap()
    lidE_in = nc.dram_tensor("lidE", [128, tcE_pad], fp16,
                             kind="ExternalInput").ap()
    lidO_in = nc.dram_tensor("lidO", [128, tcO_pad], fp16,
                             kind="ExternalInput").ap()
    w1_in = nc.dram_tensor("w1", [128, 4 * 128], fp16, kind="ExternalInput").ap()
    w2_in = nc.dram_tensor("w2", [128, 128], fp16, kind="ExternalInput").ap()
    b1_in = nc.dram_tensor("b1", [1, 128], fp16, kind="ExternalInput").ap()
    b2_in = nc.dram_tensor("b2", [1, 128], fp16, kind="ExternalInput").ap()
    rdinv_in = nc.dram_tensor("rdinv", [1, NPC], fp16, kind="ExternalInput").ap()
    dinv2_in = nc.dram_tensor("dinv2", [128, TPC], fp32, kind="ExternalInput").ap()
    dinv1_in = nc.dram_tensor("dinv1", [128, TPC], fp32, kind="ExternalInput").ap()
    pmat_in = nc.dram_tensor("pmat", [128, TPC * NG], fp16,
                             kind="ExternalInput").ap()
    iota_in = nc.dram_tensor("iota", [128, CPC * 128], fp16,
                             kind="ExternalInput").ap()
    out_dram = nc.dram_tensor("pooled", [64, 128], fp32,
                              kind="ExternalOutput").ap()

    with tile.TileContext(nc) as tcx:
        import contextlib
        ctx = contextlib.ExitStack()
        with ctx:
            dram = ctx.enter_context(tcx.tile_pool(name="dram", bufs=1, space="DRAM"))
            cpool = ctx.enter_context(tcx.tile_pool(name="const", bufs=1))
            xt_pool = ctx.enter_context(tcx.tile_pool(name="xt", bufs=4))
            g0sb_pool = ctx.enter_context(tcx.tile_pool(name="g0sb", bufs=4))
            msg_pool = ctx.enter_context(tcx.tile_pool(name="msg", bufs=8))
            s_pool = ctx.enter_context(tcx.tile_pool(name="spool", bufs=8))
            h1sb_pool = ctx.enter_context(tcx.tile_pool(name="h1sb", bufs=4))
            aggt_pool = ctx.enter_context(tcx.tile_pool(name="aggt", bufs=4))
            h2sb_pool = ctx.enter_context(tcx.tile_pool(name="h2sb", bufs=4))
            p_pool = ctx.enter_context(tcx.tile_pool(name="ppool", bufs=4))
            osb_pool = ctx.enter_context(tcx.tile_pool(name="osb", bufs=1))
            psA = ctx.enter_context(tcx.tile_pool(name="psA", bufs=4, space="PSUM"))
            psB = ctx.enter_context(tcx.tile_pool(name="psB", bufs=2, space="PSUM"))
            psP = ctx.enter_context(tcx.tile_pool(name="psP", bufs=1, space="PSUM"))

            g0_local = dram.tile([NPC, 128], fp16)
            g0_full = dram.tile([NP, 128], fp16, addr_space="Shared")
            g1_local = dram.tile([NPC, 128], fp16)
            g1_full = dram.tile([NP, 128], fp16, addr_space="Shared")

            # ---- load constants ----
            def cload(name, ap_in, shape, dt):
                t = cpool.tile(shape, dt, name=name)
                nc.sync.dma_start(out=t[:], in_=ap_in)
                return t

            idxE_sb = cload("idxE_sb", idxE_in, [128, tcE_pad * 8], i16)
            idxO_sb = cload("idxO_sb", idxO_in, [128, tcO_pad * 8], i16)
            lidE_sb = cload("lidE_sb", lidE_in, [128, tcE_pad], fp16)
            lidO_sb = cload("lidO_sb", lidO_in, [128, tcO_pad], fp16)
            w1_sb = cload("w1_sb", w1_in, [128, 4 * 128], fp16)
            w2_sb = cload("w2_sb", w2_in, [128, 128], fp16)
            b1_sb = cload("b1_sb", b1_in, [1, 128], fp16)
            b2_sb = cload("b2_sb", b2_in, [1, 128], fp16)
            rdinv_sb = cload("rdinv_sb", rdinv_in, [1, NPC], fp16)
            dinv2_sb = cload("dinv2_sb", dinv2_in, [128, TPC], fp32)
            dinv1_sb = cload("dinv1_sb", dinv1_in, [128, TPC], fp32)
            pmat_sb = cload("pmat_sb", pmat_in, [128, TPC * NG], fp16)
            iota_sb = cload("iota_sb", iota_in, [128, CPC * 128], fp16)

            # ---- g0 = (dinv*x) @ W1, node-sharded; x loaded in 2 big halves
            HT = TPC // 2
            NH = HT * 128
            for half in range(2):
                xbs = []
                for kk in range(4):
                    xb = xt_pool.tile([128, NH], fp16, tag='xb',
                                      name=f'xb_{half}_{kk}')
                    nc.sync.dma_start(
                        out=xb[:],
                        in_=xT_in[kk * 128:(kk + 1) * 128,
                                  half * NH:(half + 1) * NH])
                    xbs.append(xb)
                for ii in range(HT):
                    i = half * HT + ii
                    ps = psA.tile([128, 128], fp32, tag='agg', name=f'g0ps_{i}')
                    for kk in range(4):
                        nc.tensor.matmul(
                            ps[:], lhsT=xbs[kk][:, ii * 128:(ii + 1) * 128],
                            rhs=w1_sb[:, kk * 128:(kk + 1) * 128],
                            start=(kk == 0), stop=(kk == 3))
                    g0t = g0sb_pool.tile([128, 128], fp16)
                    nc.scalar.copy(out=g0t[:], in_=ps[:])
                    nc.sync.dma_start(out=g0_local[i * 128:(i + 1) * 128, :],
                                      in_=g0t[:])

            nc.gpsimd.collective_compute(
                "AllGather", mybir.AluOpType.bypass,
                replica_groups=[list(range(NCORES))],
                ins=[g0_local.opt()], outs=[g0_full.opt()])

            # ---- two GCN layers ----
            for layer in (1, 2):
                src_full = g0_full if layer == 1 else g1_full
                # pair view: [NP/2, 256]; window base at pair 32768
                gv = src_full[:].rearrange("(u two) d -> u (two d)", two=2)
                in_ap = {0: gv[BASE_PAIR:BASE_PAIR + UHALF, 0:128],
                         1: gv[BASE_PAIR:BASE_PAIR + UHALF, 128:256]}
                idx_sb = {0: idxE_sb, 1: idxO_sb}
                lid_sb = {0: lidE_sb, 1: lidO_sb}

                pos = {0: 0, 1: 0}            # next chunk per stream
                tiles_cur = {0: None, 1: None}
                state = {'cc': 0}
                pool_ps = None
                if layer == 2:
                    pool_ps = psP.tile([64, 128], fp32, name='poolps')

                def next_chunk(h, layer=layer, in_ap=in_ap, idx_sb=idx_sb,
                               lid_sb=lid_sb, pos=pos, tiles_cur=tiles_cur,
                               state=state):
                    p = pos[h]
                    if tiles_cur[h] is None or p % CPC == 0:
                        c0 = (p // CPC) * CPC
                        msg = msg_pool.tile([128, CPC * 128], fp16, tag='msg',
                                            name=f'msg_{layer}_{h}_{c0}')
                        nc.gpsimd.dma_gather(
                            msg[:].rearrange("p (k c) -> p k c", c=128),
                            in_ap[h],
                            idx_sb[h][:, c0 * 8:(c0 + CPC) * 8],
                            CPC * 128, CPC * 128, 128,
                            elem_step=256, single_packet=False,
                            queue_num=state['cc'] % 4)
                        state['cc'] += 1
                        sbt = s_pool.tile([128, CPC * 128], fp16, tag='s',
                                          name=f's_{layer}_{h}_{c0}')
                        nc.vector.tensor_tensor(
                            out=sbt[:].rearrange("p (k c) -> p k c", c=128),
                            in0=lid_sb[h][:, c0:c0 + CPC].to_broadcast(
                                [128, CPC, 128]),
                            in1=iota_sb[:].rearrange("p (k c) -> p k c", c=128),
                            op=iseq)
                        tiles_cur[h] = (msg, sbt, c0)
                    pos[h] = p + 1
                    msg, sbt, c0 = tiles_cur[h]
                    jj = p - c0
                    return (msg[:, jj * 128:(jj + 1) * 128],
                            sbt[:, jj * 128:(jj + 1) * 128])

                for i in range(TPC):
                    ce, co = int(cslotE[i]), int(cslotO[i])
                    ctot = ce + co
                    agg_ps = psA.tile([128, 128], fp32, tag='agg',
                                      name=f'agg_{layer}_{i}')
                    for k in range(ctot):
                        h = 0 if k < ce else 1
                        m_ap, s_ap = next_chunk(h)
                        if layer == 1:
                            nc.tensor.matmul(agg_ps[:], lhsT=s_ap, rhs=m_ap,
                                             start=(k == 0), stop=False)
                        else:
                            nc.tensor.matmul(agg_ps[:], lhsT=m_ap, rhs=s_ap,
                                             start=(k == 0),
                                             stop=(k == ctot - 1))
                    # ---- tile epilogue ----
                    rd = rdinv_sb[0:1, i * 128:(i + 1) * 128]
                    if layer == 1:
                        nc.tensor.matmul(agg_ps[:], lhsT=rd, rhs=b1_sb[0:1, :],
                                         start=(ctot == 0), stop=True)
                        h1t = h1sb_pool.tile([128, 128], fp16)
                        nc.scalar.activation(
                            out=h1t[:], in_=agg_ps[:], func=Relu,
                            scale=dinv2_sb[:, i:i + 1])
                        nc.sync.dma_start(
                            out=g1_local[i * 128:(i + 1) * 128, :], in_=h1t[:])
                    else:
                        aggt = aggt_pool.tile([128, 128], fp16)
                        nc.scalar.copy(out=aggt[:], in_=agg_ps[:])
                        h2ps = psB.tile([128, 128], fp32, tag='h2',
                                        name=f'h2ps_{i}')
                        nc.tensor.matmul(h2ps[:], lhsT=aggt[:], rhs=w2_sb[:],
                                         start=True, stop=False)
                        nc.tensor.matmul(h2ps[:], lhsT=rd, rhs=b2_sb[0:1, :],
                                         start=False, stop=True)
                        h2t = h2sb_pool.tile([128, 128], fp16)
                        nc.scalar.activation(
                            out=h2t[:], in_=h2ps[:], func=Relu,
                            scale=dinv1_sb[:, i:i + 1])
                        nc.tensor.matmul(pool_ps[:],
                                         lhsT=pmat_sb[:, i * NG:(i + 1) * NG],
                                         rhs=h2t[:],
                                         start=(i == 0), stop=(i == TPC - 1))
                if layer == 1:
                    nc.gpsimd.collective_compute(
                        "AllGather", mybir.AluOpType.bypass,
                        replica_groups=[list(range(NCORES))],
                        ins=[g1_local.opt()], outs=[g1_full.opt()])

            pooled_t = osb_pool.tile([64, 128], fp32)
            nc.scalar.copy(out=pooled_t[:], in_=pool_ps[:])
            nc.sync.dma_start(out=out_dram, in_=pooled_t[:])

    nc.compile()
    return nc


def _make_in_maps(inputs, gp):
    x = np.asarray(inputs['x'])
    W1 = np.asarray(inputs['W1'])
    b1 = np.asarray(inputs['b1'])
    W2 = np.asarray(inputs['W2'])
    b2 = np.asarray(inputs['b2'])
    dinv = gp['dinv']

    xs = np.zeros((NP, D), np.float16)
    xs[:N] = (x.astype(np.float64) * dinv[:, None]).astype(np.float16)
    w1r = np.ascontiguousarray(
        W1.astype(np.float16).reshape(4, 128, 128).transpose(1, 0, 2)
    ).reshape(128, 4 * 128)
    w2r = W2.astype(np.float16)
    b1r = b1.astype(np.float16).reshape(1, 128)
    b2r = b2.astype(np.float16).reshape(1, 128)
    iota = np.tile(np.arange(128, dtype=np.float16)[None, :], (128, CPC))

    in_maps = []
    for c in range(NCORES):
        lo, hi = c * NPC, (c + 1) * NPC
        xT = np.ascontiguousarray(xs[lo:hi].T)
        in_maps.append({
            "xT": xT,
            "idxE": gp['idxE'][c], "idxO": gp['idxO'][c],
            "lidE": gp['lidE'][c], "lidO": gp['lidO'][c],
            "w1": w1r, "w2": w2r, "b1": b1r, "b2": b2r,
            "rdinv": gp['rdinv'][lo:hi].reshape(1, NPC),
            "dinv2": gp['dinv2'][lo:hi].reshape(TPC, 128).T.copy(),
            "dinv1": gp['dinv1'][lo:hi].reshape(TPC, 128).T.copy(),
            "pmat": np.ascontiguousarray(
                gp['pmat'][lo:hi].reshape(TPC, 128, NG).transpose(1, 0, 2)
            ).reshape(128, TPC * NG),
            "iota": iota,
        })
    return in_maps


def _get_built(inputs):
    ei = np.asarray(inputs['edge_index'])
    key = hash((ei.shape, ei[0, :50].tobytes(), ei[1, -50:].tobytes()))
    if _cache.get('key') != key:
        gp = _host_prep_graph(inputs['edge_index'], inputs['batch'])
        nc = _build_nc(gp)
        _cache.update(key=key, gp=gp, nc=nc)
    return _cache['nc'], _cache['gp']


def kernel(run_kwargs=None, **inputs):
    from concourse.bass_utils import run_bass_kernel_spmd
    nc, gp = _get_built(inputs)
    in_maps = _make_in_maps(inputs, gp)
    res = run_bass_kernel_spmd(nc, in_maps, list(range(NCORES)),
                               **(run_kwargs or {}))
    out = np.zeros((64, 128), np.float64)
    for r in res.results:
        out += r["pooled"].astype(np.float64)
    if run_kwargs:
        _cache['last_res'] = res
    return out.astype(np.float32)
